# revision 35
# baseline (speedup 1.0000x reference)
"""Trainium2 Bass kernel for one dense transformer block (RMSNorm -> causal
RoPE attention -> residual -> RMSNorm -> GELU MLP -> residual).

Sharding across 8 NeuronCores: 2 batch-groups (data parallel over B=2) x 4
ranks. Within a group: tensor-parallel over heads for QKV+attention, 4-rank
ReduceScatter reshards the out_proj partial sums to sequence-parallel, then
each rank runs the MLP on its own 512-token shard with full weights.

v5 structure (from trace-driven iteration; v2 baseline was ~956us):
- DMA queue discipline: SP carries only never-waiting weight/x streams and
  output stores; out_proj->rs_in stores go on the ACT queue; the RS-gated
  rs_out loads go on the GpSimd queue (so no in-order queue mixes an
  RS-gated op with work another phase needs -- the v2 bottleneck).
- Half-0 residual tiles live in a dedicated virgin-SBUF pool so their loads
  are not zone-WAR-blocked behind phase-B tile readers.
- wfc/wproj stream pools are placed (via open order + a pad pool) in phase
  A's dead SBUF zone: their prefetch fills every ring slot during phase B,
  riding out the DMA contention with the in-flight ReduceScatter.
- MLP split into sequence-halves; half 0's FC+proj hides RS1.
- Per-engine ordering edges at phase seams, targeted a few tiles before the
  seam so the next phase's lead-in chain overlaps the previous phase tail.
- exp batched over key-block pairs; psum->sbuf copies on DVE; rms stats via
  fp16 square + ones-matmul partition broadcast (no DRAM roundtrip).
- fp8 evaluated and rejected: any single fp8 matmul costs 1-2.8e-2 max-rel
  error vs the 2e-2 budget.
"""

import os
import sys

import numpy as np

for _p in ("/root/.axon_site/_ro/trn_rl_repo", "/opt/trn_rl_repo"):
    if os.path.isdir(_p) and _p not in sys.path:
        sys.path.append(_p)

import concourse.bass as bass  # noqa: E402
import concourse.mybir as mybir  # noqa: E402
import concourse.tile as tile  # noqa: E402
from concourse import bacc  # noqa: E402
from concourse.bass_utils import run_bass_kernel_spmd  # noqa: E402
from concourse.tile import add_dep_helper  # noqa: E402

F32 = mybir.dt.float32
F16 = mybir.dt.float16
AF = mybir.ActivationFunctionType

G = 4  # ranks per batch-group
NCORES = 8
DH = 128  # head dim (= partition width)
EPS = 1e-6
ROPE_BASE = 10000.0
EXPB = -3.0  # softmax exp bias


def build_nc(S, D, H, FF, gelu=None):
    gelu = gelu if gelu is not None else AF.Gelu_apprx_tanh
    HC = H // G  # heads per core
    SC = S // 4  # chunk length == sequence shard length
    SH = SC // 2  # q-half length (RS pipeline granularity)
    ND = D // 128
    NQK = 2 * HC
    NV = HC * DH
    NKBC = SC // 128  # 128-token k-blocks per chunk
    NFF = FF // 128
    ISQ = float(1.0 / np.sqrt(DH))
    RSDT = F16

    nc = bacc.Bacc("TRN2", target_bir_lowering=False, debug=False, num_devices=NCORES)

    xT_in = nc.dram_tensor("xT", [4, 128, ND, SC], F16, kind="ExternalInput")
    xrT_in = nc.dram_tensor("xrT", [128, ND, SC], F16, kind="ExternalInput")
    wqk_in = nc.dram_tensor("wqk", [NQK, 128, ND, 128], F16, kind="ExternalInput")
    wv_in = nc.dram_tensor("wv", [128, ND, NV], F16, kind="ExternalInput")
    wout_in = nc.dram_tensor("wout", [128, HC, D], F16, kind="ExternalInput")
    wfc_in = nc.dram_tensor("wfc", [NFF, 128, ND, 128], F16, kind="ExternalInput")
    wproj_in = nc.dram_tensor("wproj", [ND, 128, NFF, 128], F16, kind="ExternalInput")
    ctab_in = nc.dram_tensor("ctab", [64, S], F32, kind="ExternalInput")
    stab_in = nc.dram_tensor("stab", [64, S], F32, kind="ExternalInput")
    mask_in = nc.dram_tensor("masks", [128, 2, SH], F32, kind="ExternalInput")
    ones_in = nc.dram_tensor("ones", [128, 128], F16, kind="ExternalInput")
    out_ext = nc.dram_tensor("out", [ND, 128, SC], F16, kind="ExternalOutput")

    with tile.TileContext(nc) as tc:
        with (
            tc.tile_pool(name="const", bufs=1) as constp,
            tc.tile_pool(name="dram", bufs=1, space="DRAM") as dramp,
            tc.tile_pool(name="d_early", bufs=1) as dearly,
            tc.tile_pool(name="d_wk_e", bufs=2) as dwke,
        ):
            rs_in_s = [dramp.tile([G, 128, ND, SH], RSDT, name=f"rsi{s}",
                                  tag=f"rsi{s}") for s in range(2)]
            rs_out_s = [dramp.tile([128, ND, SH], RSDT, name=f"rso{s}",
                                   tag=f"rso{s}") for s in range(2)]

            ones_sb = constp.tile([128, 128], F16)
            nc.sync.dma_start(ones_sb[:], ones_in[:])
            eps_sb = constp.tile([1, 1], F32)
            nc.vector.memset(eps_sb[:], EPS)
            nb3_sb = constp.tile([128, 1], F32)
            nc.vector.memset(nb3_sb[:], EXPB)
            masks = constp.tile([128, 2, SH], F32)
            wout_sb = constp.tile([128, HC, D], F16)

            # half-0 residual tiles in virgin SBUF: no zone-reuse WAR against
            # A/B tiles, so their loads/writes can run during phases A/B
            xr0_sb = dearly.tile([128, ND, SH], F16)
            rsb0_sb = dearly.tile([128, ND, SH], RSDT)
            out10_sb = dearly.tile([128, ND, SH], F16)
            h2T0_sb = dearly.tile([128, ND, SH], F16)

            # q/k/v pool scoped to phases A+B; phase D reuses this SBUF
            kvq_ctx = tc.tile_pool(name="kvq", bufs=1)
            kvqp = kvq_ctx.__enter__()
            krT = kvqp.tile([128, HC, S], F16)
            q_sb = kvqp.tile([128, HC, S], F16)
            vtok = kvqp.tile([128, S // 128, NV], F16)

            # ================= phase A: stats + QKV + RoPE =================
            with (
                tc.tile_pool(name="a_w", bufs=1) as awp,
                tc.tile_pool(name="a_str", bufs=3) as astr,
                tc.tile_pool(name="a_x", bufs=2) as axp,
                tc.tile_pool(name="a_work", bufs=2) as awk,
                tc.tile_pool(name="a_ps", bufs=1, space="PSUM") as apsum,
            ):
                # startup-critical loads first: x(0) in d-quarters so the
                # stats chain starts after the first 512KB, then rope tables
                x_tiles = [None] * 4
                x_tiles[0] = axp.tile([128, ND, SC], F16, tag="xchunk",
                                      name="xc0")
                for dd in range(0, ND, 4):
                    nc.sync.dma_start(x_tiles[0][:, dd:dd + 4, :],
                                      xT_in[0][:, dd:dd + 4, :])
                wqk0 = astr.tile([128, ND, 128], F16, tag="wqk", name="wqk0")
                nc.sync.dma_start(wqk0[:], wqk_in[0])
                ctab_sb = awp.tile([64, S], F32)
                stab_sb = awp.tile([64, S], F32)
                nc.sync.dma_start(ctab_sb[:], ctab_in[:])
                nc.sync.dma_start(stab_sb[:], stab_in[:])

                def stats(c, scale_x=True):
                    """Compute 1/rms for chunk c; optionally scale x in place.
                    Returns the [128, SC] psum broadcast of 1/rms."""
                    if c > 0:
                        x_tiles[c] = axp.tile([128, ND, SC], F16, tag="xchunk",
                                              name=f"xc{c}")
                        for dd in range(0, ND, 4):
                            nc.sync.dma_start(x_tiles[c][:, dd:dd + 4, :],
                                              xT_in[c][:, dd:dd + 4, :])
                    x_sb = x_tiles[c]
                    ps_ss = apsum.tile([1, SC], F32, tag="ss", bufs=2)
                    for dd in range(0, ND, 4):
                        xsq = awk.tile([128, 4, SC], F16, tag="xsq")
                        nc.scalar.square(xsq[:], x_sb[:, dd:dd + 4, :])
                        for d in range(4):
                            nc.tensor.matmul(
                                ps_ss[:], ones_sb[:, 0:1], xsq[:, d, :],
                                start=(dd + d == 0), stop=(dd + d == ND - 1),
                            )
                    rcp = awk.tile([1, SC], F16, tag="rcp")
                    nc.scalar.activation(
                        rcp[:], ps_ss[:], AF.Sqrt, bias=eps_sb[:], scale=1.0 / D
                    )
                    with nc.allow_low_precision(reason="1/rms fp16 is plenty"):
                        nc.vector.reciprocal(rcp[:], rcp[:])
                    ps_rb = apsum.tile([128, SC], F32, tag="rb", bufs=1)
                    nc.tensor.matmul(
                        ps_rb[:], ones_sb[0:1, :], rcp[:], start=True, stop=True
                    )
                    if scale_x:
                        for d in range(ND):
                            nc.vector.tensor_mul(
                                x_sb[:, d, :], x_sb[:, d, :], ps_rb[:]
                            )
                    return ps_rb

                # chunk 0: QK runs on RAW x with 1/rms folded into the rope
                # tables, so the first matmuls start right after wqk[0] lands
                ps_rb0 = stats(0, scale_x=False)
                ctr0 = awk.tile([64, SC], F32, tag="ctr0")
                srt0 = awk.tile([64, SC], F32, tag="srt0")
                nc.vector.tensor_mul(ctr0[:], ctab_sb[:, 0:SC], ps_rb0[0:64, :])
                nc.vector.tensor_mul(srt0[:], stab_sb[:, 0:SC], ps_rb0[0:64, :])

                wv_sb = awp.tile([128, ND, NV], F16)
                for c in range(4):
                    csl = slice(c * SC, (c + 1) * SC)
                    x_sb = x_tiles[c]
                    ct = ctr0 if c == 0 else ctab_sb[:, csl]
                    st = srt0 if c == 0 else stab_sb[:, csl]

                    for m in range(NQK):
                        if c == 0 and m == 0:
                            wm = wqk0
                        else:
                            wm = astr.tile([128, ND, 128], F16, tag="wqk")
                            nc.sync.dma_start(wm[:], wqk_in[m])
                        ps_qk = apsum.tile([128, SC], F32, tag="qk", bufs=3)
                        for d in range(ND):
                            nc.tensor.matmul(
                                ps_qk[:], wm[:, d, :], x_sb[:, d, :],
                                start=(d == 0), stop=(d == ND - 1),
                            )
                        if m < HC:
                            ro = q_sb[:, m, csl]
                        else:
                            ro = krT[:, m - HC, csl]
                        t1 = awk.tile([64, SC], F32, tag="t1")
                        t2 = awk.tile([64, SC], F32, tag="t2")
                        nc.vector.tensor_mul(t1[:], ps_qk[0:64, :], ct[:])
                        nc.vector.tensor_mul(t2[:], ps_qk[64:128, :], st[:])
                        nc.vector.tensor_sub(ro[0:64, :], t1[:], t2[:])
                        nc.vector.tensor_mul(t1[:], ps_qk[64:128, :], ct[:])
                        nc.vector.tensor_mul(t2[:], ps_qk[0:64, :], st[:])
                        nc.vector.tensor_add(ro[64:128, :], t1[:], t2[:])

                    if c == 0:
                        # V needs scaled x: do the deferred in-place scale now
                        for d in range(ND):
                            nc.vector.tensor_mul(
                                x_sb[:, d, :], x_sb[:, d, :], ps_rb0[:]
                            )
                        nc.sync.dma_start(wv_sb[:], wv_in[:])
                    elif c == 1:
                        nc.sync.dma_start(masks[:], mask_in[:])
                    elif c == 2:
                        nc.sync.dma_start(wout_sb[:], wout_in[:])
                        nc.sync.dma_start(xr0_sb[:], xrT_in[:, :, 0:SH])
                    if c < 3:
                        stats(c + 1)

                    for sb in range(NKBC):
                        ps_v = apsum.tile([128, NV], F32, tag="v", bufs=2)
                        tsl = slice(sb * 128, (sb + 1) * 128)
                        for d in range(ND):
                            nc.tensor.matmul(
                                ps_v[:], x_sb[:, d, tsl], wv_sb[:, d, :],
                                start=(d == 0), stop=(d == ND - 1),
                            )
                        nc.vector.tensor_copy(vtok[:, c * NKBC + sb, :], ps_v[:])

            # ========== phase B: attention + fused out_proj partials ==========
            # two q-halves; each half's out_proj partials feed their own RS
            last_b = {}  # instruction handles for cross-phase ordering edges
            cc_h = [None, None]
            with (
                tc.tile_pool(name="b_work", bufs=2) as bwk,
                tc.tile_pool(name="b_pt", bufs=3) as bpt,
                tc.tile_pool(name="b_av", bufs=2) as bav,
                tc.tile_pool(name="b_ost", bufs=2) as bost,
                tc.tile_pool(name="b_ps", bufs=2, space="PSUM") as bps,
                tc.tile_pool(name="b_ps_acc", bufs=2, space="PSUM") as bpsa,
                tc.tile_pool(name="b_ps_den", bufs=2, space="PSUM") as bpsd,
                tc.tile_pool(name="b_ps_op", bufs=2, space="PSUM") as bpso,
            ):
                for c in range(4):
                    for s in range(2):
                        qsl = slice(c * SC + s * SH, c * SC + (s + 1) * SH)
                        nkb = 4 * c + 2 * s + 2
                        npair = nkb // 2
                        avc = bav.tile([128, HC, SH], F16, tag="avc")
                        for h in range(HC):
                            ps_av = bpsa.tile([128, SH], F32, tag="av")
                            ps_den = bpsd.tile([128, SH], F32, tag="dn")
                            prev = None

                            def flush(prev):
                                pp, pi = prev
                                for j in range(2):
                                    kb = 2 * pi + j
                                    nc.tensor.matmul(
                                        ps_av[:],
                                        vtok[:, kb, h * DH:(h + 1) * DH],
                                        pp[:, j, :], start=(kb == 0),
                                        stop=(kb == nkb - 1),
                                    )
                                    nc.tensor.matmul(
                                        ps_den[:], ones_sb[:], pp[:, j, :],
                                        start=(kb == 0), stop=(kb == nkb - 1),
                                    )

                            for pi in range(npair):
                                pts = bpt.tile([128, 2, SH], F16, tag="pt")
                                ps_sc = bps.tile([128, 2, SH], F32, tag="sc")
                                for j in range(2):
                                    kb = 2 * pi + j
                                    nc.tensor.matmul(
                                        ps_sc[:, j, :],
                                        krT[:, h, kb * 128:(kb + 1) * 128],
                                        q_sb[:, h, qsl],
                                        start=True, stop=True,
                                    )
                                if pi == npair - 1:
                                    nc.vector.tensor_add(
                                        ps_sc[:], ps_sc[:], masks[:]
                                    )
                                nc.scalar.activation(
                                    pts[:], ps_sc[:], AF.Exp,
                                    bias=nb3_sb[:], scale=ISQ,
                                )
                                if prev is not None:
                                    flush(prev)
                                prev = (pts, pi)
                            flush(prev)
                            denb = bwk.tile([128, SH], F32, tag="denb")
                            nc.vector.reciprocal(denb[:], ps_den[:])
                            nc.vector.tensor_mul(avc[:, h, :], ps_av[:], denb[:])
                        # fused out_proj partials for this q-half
                        ost = bost.tile([128, ND, SH], RSDT, tag="ost")
                        for m in range(ND):
                            ps_op = bpso.tile([128, SH], F32, tag="op")
                            for fb in range(HC):
                                last_b["pe"] = nc.tensor.matmul(
                                    ps_op[:],
                                    wout_sb[:, fb, m * 128:(m + 1) * 128],
                                    avc[:, fb, :],
                                    start=(fb == 0), stop=(fb == HC - 1),
                                )
                            last_b["dve"] = nc.vector.tensor_copy(
                                ost[:, m, :], ps_op[:]
                            )
                        # single batched store on the ACT queue (SP queue must
                        # stay free of anything phase B produces/consumes).
                        # chunk-pair RS split: RS0 carries chunks 0-1 (pieces
                        # 2c+s) and fires ~28% into phase B; RS1 carries
                        # chunks 2-3.
                        grp = c // 2
                        last_b["actq"] = nc.scalar.dma_start(
                            rs_in_s[grp][2 * (c % 2) + s], ost[:]
                        )
                        if c == 2 and s == 1:
                            # relaxed B->D fence: phase D's ACT/DVE lead-in may
                            # interleave with the final c3 tiles; RS0 is long
                            # done by then
                            last_b["dve_c2"] = last_b["dve"]
                            last_b["actq_c2"] = last_b["actq"]
                        if c % 2 == 1 and s == 1:
                            cc_h[grp] = nc.gpsimd.collective_compute(
                                "ReduceScatter",
                                mybir.AluOpType.add,
                                replica_groups=[[0, 1, 2, 3], [4, 5, 6, 7]],
                                ins=[rs_in_s[grp][:].opt()],
                                outs=[rs_out_s[grp][:].opt()],
                            )

            kvq_ctx.__exit__(None, None, None)

            # ============ phase D: residual + RMSNorm2 + MLP ==============
            # split into sequence-halves: half 0's FC+proj hides RS1.
            # pool open order + pad place wfc/wproj streams in phase-A's dead
            # SBUF zone so their prefetch fills every slot during phase B.
            with (
                tc.tile_pool(name="d_res", bufs=1) as dres,
                tc.tile_pool(name="d_pad", bufs=1) as dpad,
                tc.tile_pool(name="d_wfc", bufs=10) as dwfc,
                tc.tile_pool(name="d_wpr", bufs=4) as dwpr,
                tc.tile_pool(name="d_o2", bufs=8) as do2,
                tc.tile_pool(name="d_ps", bufs=2, space="PSUM") as dps,
                tc.tile_pool(name="d_ps1", bufs=2, space="PSUM") as dps1,
            ):
                pad = dpad.tile([128, 5120], F16)  # keep streams off B's zone
                gT_tiles = [
                    dres.tile([128, NFF, SH], F16, name=f"gT{s}", tag=f"gT{s}")
                    for s in range(2)
                ]
                xr1_sb = dres.tile([128, ND, SH], F16)
                nc.sync.dma_start(xr1_sb[:], xrT_in[:, :, SH:SC])
                first_d = {}
                prev_s = {}
                NPRE = 10
                for s in range(2):
                    ssl = slice(s * SH, (s + 1) * SH)
                    if s == 0:
                        xr, rsb, out1, h2T = xr0_sb, rsb0_sb, out10_sb, h2T0_sb
                    else:
                        xr = xr1_sb
                        rsb = dres.tile([128, ND, SH], RSDT, tag="rsb1")
                        out1 = dres.tile([128, ND, SH], F16, tag="out11")
                        h2T = dres.tile([128, ND, SH], F16, tag="h2T1")
                    # wfc prefetch: fill all ring slots before FC needs them
                    wms = [None] * NFF

                    def load_wfc(m, s=s):
                        wms[m] = dwfc.tile([128, ND, 128], F16, tag="wfc",
                                           name=f"wfc{s}_{m}")
                        nc.sync.dma_start(wms[m][:], wfc_in[m])

                    for m in range(NPRE):
                        load_wfc(m)

                    # RS-gated load on the GpSimd queue: nothing else needs it
                    ld = nc.gpsimd.dma_start(rsb[:], rs_out_s[s][:])
                    if s == 0:
                        # keep RS1's trigger behind the rsb0 load on the
                        # gpsimd queue (it can't fire earlier anyway)
                        add_dep_helper(cc_h[1].ins, ld.ins,
                                       reason="rs1 trigger after rsb0 load")
                    h = nc.vector.tensor_add(out1[:], rsb[:], xr[:])
                    if s == 0:
                        first_d["dve"] = h
                    else:
                        add_dep_helper(h.ins, prev_s["dve"].ins,
                                       reason="D half order dve")
                    ps_ss2 = dps1.tile([1, SH], F32, tag="ss2")
                    for dd in range(0, ND, 4):
                        xsq = dwke.tile([128, 4, SH], F16, tag="xsq2")
                        h = nc.scalar.square(xsq[:], out1[:, dd:dd + 4, :])
                        if s == 0 and dd == 0:
                            first_d["act"] = h
                        elif s == 1 and dd == 0:
                            add_dep_helper(h.ins, prev_s["act"].ins,
                                           reason="D half order act")
                        for d in range(4):
                            h = nc.tensor.matmul(
                                ps_ss2[:], ones_sb[:, 0:1], xsq[:, d, :],
                                start=(dd + d == 0), stop=(dd + d == ND - 1),
                            )
                            if s == 0 and dd + d == 0:
                                first_d["pe"] = h
                            elif s == 1 and dd + d == 0:
                                add_dep_helper(h.ins, prev_s["pe"].ins,
                                               reason="D half order pe")
                    rcp2 = dwke.tile([1, SH], F16, tag="rcp2")
                    nc.scalar.activation(
                        rcp2[:], ps_ss2[:], AF.Sqrt, bias=eps_sb[:], scale=1.0 / D
                    )
                    with nc.allow_low_precision(reason="1/rms fp16 is plenty"):
                        nc.vector.reciprocal(rcp2[:], rcp2[:])
                    ps_rb2 = dps1.tile([128, SH], F32, tag="rb2")
                    nc.tensor.matmul(
                        ps_rb2[:], ones_sb[0:1, :], rcp2[:], start=True, stop=True
                    )
                    for d in range(ND):
                        nc.vector.tensor_mul(h2T[:, d, :], out1[:, d, :], ps_rb2[:])

                    gT = gT_tiles[s]
                    for m in range(NFF):
                        ps_fc = dps.tile([128, SH], F32, tag="fc")
                        for d in range(ND):
                            nc.tensor.matmul(
                                ps_fc[:], wms[m][:, d, :], h2T[:, d, :],
                                start=(d == 0), stop=(d == ND - 1),
                            )
                        if m + NPRE < NFF:
                            load_wfc(m + NPRE)
                        gl = nc.scalar.activation(gT[:, m, :], ps_fc[:], gelu)

                    wps = [None] * ND

                    def load_wpr(m):
                        wps[m] = dwpr.tile([128, NFF, 128], F16, tag="wproj",
                                           name=f"wpr{s}_{m}")
                        nc.sync.dma_start(wps[m][:], wproj_in[m])

                    for m in range(4):
                        load_wpr(m)
                    for m in range(ND):
                        ps_pr = dps.tile([128, SH], F32, tag="pr")
                        for k in range(NFF):
                            pe = nc.tensor.matmul(
                                ps_pr[:], wps[m][:, k, :], gT[:, k, :],
                                start=(k == 0), stop=(k == NFF - 1),
                            )
                        if m + 4 < ND:
                            load_wpr(m + 4)
                        o2 = do2.tile([128, SH], F16, tag="o2")
                        dve = nc.vector.tensor_add(o2[:], ps_pr[:], out1[:, m, :])
                        nc.sync.dma_start(out_ext[m][:, ssl], o2[:])
                        if s == 0 and m == 10:
                            # half-1 lead-in may overlap the proj(0) tail:
                            # RS1 is long done by proj(0) m=10
                            mid_dve = dve
                    prev_s = {"pe": pe, "dve": mid_dve if s == 0 else dve,
                              "act": gl}

                # B->D ordering edges: nothing phase-D (all RS-gated) may be
                # emitted ahead of phase-B work in any engine queue
                add_dep_helper(first_d["pe"].ins, last_b["pe"].ins,
                               reason="D after B: tensor queue")
                add_dep_helper(first_d["dve"].ins, last_b["dve_c2"].ins,
                               reason="D after B: vector queue")
                add_dep_helper(first_d["act"].ins, last_b["actq_c2"].ins,
                               reason="D after B: scalar queue")

    nc.compile()
    return nc


def _deinterleave(w):
    """Reorder head-dim columns: evens then odds (per 128-wide head)."""
    Din, Dout = w.shape
    nh = Dout // DH
    w4 = w.reshape(Din, nh, DH // 2, 2)
    return np.concatenate([w4[..., 0], w4[..., 1]], axis=2).reshape(Din, Dout)


def prep_inputs(x, w_qkv, w_out, w_fc, w_proj, g_in, g_ff, S, D, H, FF):
    HC = H // G
    SC = S // 4
    SH = SC // 2
    ND = D // 128
    NQK = 2 * HC
    NV = HC * DH
    NFF = FF // 128

    x = np.asarray(x, np.float32)
    w_qkv = np.asarray(w_qkv, np.float32)
    w_out = np.asarray(w_out, np.float32)
    w_fc = np.asarray(w_fc, np.float32)
    w_proj = np.asarray(w_proj, np.float32)
    g_in = np.asarray(g_in, np.float32)
    g_ff = np.asarray(g_ff, np.float32)

    wq = w_qkv * g_in[:, None]

    half = DH // 2
    invf = 1.0 / (ROPE_BASE ** (2.0 * np.arange(half, dtype=np.float64) / DH))
    ang = np.arange(S, dtype=np.float64)[:, None] * invf[None, :]
    ctab = np.ascontiguousarray(np.cos(ang).T.astype(np.float32))
    stab = np.ascontiguousarray(np.sin(ang).T.astype(np.float32))

    # two per-half mask patterns: key-block at offset 0 / 128 below the
    # q-half base (q-half rows t=0..SH-1 attend keys <= base+t)
    pp = np.arange(128)[:, None]
    tt = np.arange(SH)[None, :]
    masks = np.stack([
        np.where(pp <= tt, 0.0, -1000.0),
        np.where(pp <= tt - 128, 0.0, -1000.0),
    ], axis=0).astype(np.float32)
    masks = np.ascontiguousarray(masks.transpose(1, 0, 2))

    wfc = w_fc * g_ff[:, None]
    wfc_r = np.ascontiguousarray(
        wfc.reshape(ND, 128, NFF, 128).transpose(2, 1, 0, 3)
    ).astype(np.float16)
    wproj_r = np.ascontiguousarray(
        w_proj.reshape(NFF, 128, ND, 128).transpose(2, 1, 0, 3)
    ).astype(np.float16)
    ones16 = np.ones((128, 128), np.float16)

    in_maps = []
    for core in range(NCORES):
        b, t = core // G, core % G
        xb = x[b]
        xT = np.ascontiguousarray(xb.T)
        xT_r = np.ascontiguousarray(
            xT.reshape(ND, 128, 4, SC).transpose(2, 1, 0, 3)
        ).astype(np.float16)

        # chunk-pair RS mapping: this rank's D-half h covers global tokens
        # (2h + t//2)*SC + (t%2)*SH .. +SH
        def seg(lo):
            return xb[lo:lo + SH, :].T.reshape(ND, 128, SH).transpose(1, 0, 2)

        g0 = (t // 2) * SC + (t % 2) * SH
        g1 = (2 + t // 2) * SC + (t % 2) * SH
        xrT = np.ascontiguousarray(
            np.concatenate([seg(g0), seg(g1)], axis=2)
        ).astype(np.float16)
        qcols = _deinterleave(wq[:, t * NV:(t + 1) * NV])
        kcols = _deinterleave(wq[:, D + t * NV:D + (t + 1) * NV])
        vcols = wq[:, 2 * D + t * NV:2 * D + (t + 1) * NV]
        wqk_core = np.ascontiguousarray(
            np.concatenate([qcols, kcols], axis=1)
            .reshape(ND, 128, NQK, 128).transpose(2, 1, 0, 3)
        ).astype(np.float16)
        wv_core = np.ascontiguousarray(
            vcols.reshape(ND, 128, NV).transpose(1, 0, 2)
        ).astype(np.float16)
        wout_core = np.ascontiguousarray(
            w_out[t * NV:(t + 1) * NV, :].reshape(HC, 128, D).transpose(1, 0, 2)
        ).astype(np.float16)
        in_maps.append({
            "xT": xT_r, "xrT": xrT, "wqk": wqk_core, "wv": wv_core,
            "wout": wout_core, "wfc": wfc_r, "wproj": wproj_r,
            "ctab": ctab, "stab": stab, "masks": masks, "ones": ones16,
        })
    return in_maps


def assemble(results, S, D):
    SC = S // 4
    SH = SC // 2
    y = np.zeros((2, S, D), np.float32)
    for core in range(NCORES):
        b, t = core // G, core % G
        o = results[core]["out"].reshape(D, SC)
        g0 = (t // 2) * SC + (t % 2) * SH
        g1 = (2 + t // 2) * SC + (t % 2) * SH
        y[b, g0:g0 + SH, :] = o[:, :SH].T.astype(np.float32)
        y[b, g1:g1 + SH, :] = o[:, SH:].T.astype(np.float32)
    return y


_CACHE = {}


def run(inputs, S, D, H, FF, trace=False, **kw):
    key = (S, D, H, FF)
    if key not in _CACHE:
        _CACHE[key] = build_nc(S, D, H, FF)
    nc = _CACHE[key]
    in_maps = prep_inputs(
        inputs["x"], inputs["w_qkv"], inputs["w_out"], inputs["w_fc"],
        inputs["w_proj"], inputs["g_in"], inputs["g_ff"], S, D, H, FF,
    )
    res = run_bass_kernel_spmd(nc, in_maps, list(range(NCORES)), trace=trace, **kw)
    return assemble(res.results, S, D), res


def kernel(**inputs):
    y, _ = run(inputs, S=2048, D=2048, H=16, FF=4096)
    return y.astype(np.float32)


# revision 36
# speedup vs baseline: 1.0267x; 1.0267x over previous
"""Trainium2 Bass kernel for one dense transformer block (RMSNorm -> causal
RoPE attention -> residual -> RMSNorm -> GELU MLP -> residual).

Sharding across 8 NeuronCores: 2 batch-groups (data parallel over B=2) x 4
ranks. Within a group: tensor-parallel over heads for QKV+attention, 4-rank
ReduceScatter reshards the out_proj partial sums to sequence-parallel, then
each rank runs the MLP on its own 512-token shard with full weights.

v5 structure (from trace-driven iteration; v2 baseline was ~956us):
- DMA queue discipline: SP carries only never-waiting weight/x streams and
  output stores; out_proj->rs_in stores go on the ACT queue; the RS-gated
  rs_out loads go on the GpSimd queue (so no in-order queue mixes an
  RS-gated op with work another phase needs -- the v2 bottleneck).
- Half-0 residual tiles live in a dedicated virgin-SBUF pool so their loads
  are not zone-WAR-blocked behind phase-B tile readers.
- wfc/wproj stream pools are placed (via open order + a pad pool) in phase
  A's dead SBUF zone: their prefetch fills every ring slot during phase B,
  riding out the DMA contention with the in-flight ReduceScatter.
- MLP split into sequence-halves; half 0's FC+proj hides RS1.
- Per-engine ordering edges at phase seams, targeted a few tiles before the
  seam so the next phase's lead-in chain overlaps the previous phase tail.
- exp batched over key-block pairs; psum->sbuf copies on DVE; rms stats via
  fp16 square + ones-matmul partition broadcast (no DRAM roundtrip).
- fp8 evaluated and rejected: any single fp8 matmul costs 1-2.8e-2 max-rel
  error vs the 2e-2 budget.
"""

import os
import sys

import numpy as np

for _p in ("/root/.axon_site/_ro/trn_rl_repo", "/opt/trn_rl_repo"):
    if os.path.isdir(_p) and _p not in sys.path:
        sys.path.append(_p)

import concourse.bass as bass  # noqa: E402
import concourse.mybir as mybir  # noqa: E402
import concourse.tile as tile  # noqa: E402
from concourse import bacc  # noqa: E402
from concourse.bass_utils import run_bass_kernel_spmd  # noqa: E402
from concourse.tile import add_dep_helper  # noqa: E402

F32 = mybir.dt.float32
F16 = mybir.dt.float16
AF = mybir.ActivationFunctionType

G = 4  # ranks per batch-group
NCORES = 8
DH = 128  # head dim (= partition width)
EPS = 1e-6
ROPE_BASE = 10000.0
EXPB = -3.0  # softmax exp bias


def build_nc(S, D, H, FF, gelu=None):
    gelu = gelu if gelu is not None else AF.Gelu_apprx_tanh
    HC = H // G  # heads per core
    SC = S // 4  # chunk length == sequence shard length
    SH = SC // 2  # q-half length (RS pipeline granularity)
    ND = D // 128
    NQK = 2 * HC
    NV = HC * DH
    NKBC = SC // 128  # 128-token k-blocks per chunk
    NFF = FF // 128
    ISQ = float(1.0 / np.sqrt(DH))
    RSDT = F16

    nc = bacc.Bacc("TRN2", target_bir_lowering=False, debug=False, num_devices=NCORES)

    xT_in = nc.dram_tensor("xT", [4, 128, ND, SC], F16, kind="ExternalInput")
    xrT_in = nc.dram_tensor("xrT", [128, ND, SC], F16, kind="ExternalInput")
    wqk_in = nc.dram_tensor("wqk", [NQK, 128, ND, 128], F16, kind="ExternalInput")
    wv_in = nc.dram_tensor("wv", [128, ND, NV], F16, kind="ExternalInput")
    wout_in = nc.dram_tensor("wout", [128, HC, D], F16, kind="ExternalInput")
    wfc_in = nc.dram_tensor("wfc", [NFF, 128, ND, 128], F16, kind="ExternalInput")
    wproj_in = nc.dram_tensor("wproj", [ND, 128, NFF, 128], F16, kind="ExternalInput")
    ctab_in = nc.dram_tensor("ctab", [64, S], F32, kind="ExternalInput")
    stab_in = nc.dram_tensor("stab", [64, S], F32, kind="ExternalInput")
    mask_in = nc.dram_tensor("masks", [128, 2, SH], F32, kind="ExternalInput")
    ones_in = nc.dram_tensor("ones", [128, 128], F16, kind="ExternalInput")
    out_ext = nc.dram_tensor("out", [ND, 128, SC], F16, kind="ExternalOutput")

    with tile.TileContext(nc) as tc:
        with (
            tc.tile_pool(name="const", bufs=1) as constp,
            tc.tile_pool(name="dram", bufs=1, space="DRAM") as dramp,
            tc.tile_pool(name="d_early", bufs=1) as dearly,
            tc.tile_pool(name="d_wk_e", bufs=2) as dwke,
        ):
            rs_in_s = [dramp.tile([G, 128, ND, SH], RSDT, name=f"rsi{s}",
                                  tag=f"rsi{s}") for s in range(2)]
            rs_out_s = [dramp.tile([128, ND, SH], RSDT, name=f"rso{s}",
                                   tag=f"rso{s}") for s in range(2)]

            ones_sb = constp.tile([128, 128], F16)
            nc.sync.dma_start(ones_sb[:], ones_in[:])
            eps_sb = constp.tile([1, 1], F32)
            nc.vector.memset(eps_sb[:], EPS)
            nb3_sb = constp.tile([128, 1], F32)
            nc.vector.memset(nb3_sb[:], EXPB)
            masks = constp.tile([128, 2, SH], F32)
            wout_sb = constp.tile([128, HC, D], F16)

            # half-0 residual tiles in virgin SBUF: no zone-reuse WAR against
            # A/B tiles, so their loads/writes can run during phases A/B
            xr0_sb = dearly.tile([128, ND, SH], F16)
            rsb0_sb = dearly.tile([128, ND, SH], RSDT)
            out10_sb = dearly.tile([128, ND, SH], F16)
            h2T0_sb = dearly.tile([128, ND, SH], F16)

            # q/k/v pool scoped to phases A+B; phase D reuses this SBUF
            kvq_ctx = tc.tile_pool(name="kvq", bufs=1)
            kvqp = kvq_ctx.__enter__()
            krT = kvqp.tile([128, HC, S], F16)
            q_sb = kvqp.tile([128, HC, S], F16)
            vtok = kvqp.tile([128, S // 128, NV], F16)

            # ================= phase A: stats + QKV + RoPE =================
            with (
                tc.tile_pool(name="a_w", bufs=1) as awp,
                tc.tile_pool(name="a_str", bufs=3) as astr,
                tc.tile_pool(name="a_x", bufs=2) as axp,
                tc.tile_pool(name="a_work", bufs=2) as awk,
                tc.tile_pool(name="a_ps", bufs=1, space="PSUM") as apsum,
            ):
                # startup-critical loads first: x(0) in d-quarters so the
                # stats chain starts after the first 512KB, then rope tables
                x_tiles = [None] * 4
                x_tiles[0] = axp.tile([128, ND, SC], F16, tag="xchunk",
                                      name="xc0")
                for dd in range(0, ND, 4):
                    nc.sync.dma_start(x_tiles[0][:, dd:dd + 4, :],
                                      xT_in[0][:, dd:dd + 4, :])
                wqk0 = astr.tile([128, ND, 128], F16, tag="wqk", name="wqk0")
                nc.sync.dma_start(wqk0[:], wqk_in[0])
                ctab_sb = awp.tile([64, S], F32)
                stab_sb = awp.tile([64, S], F32)
                nc.sync.dma_start(ctab_sb[:], ctab_in[:])
                nc.sync.dma_start(stab_sb[:], stab_in[:])

                def stats(c, scale_x=True):
                    """Compute 1/rms for chunk c; optionally scale x in place.
                    Returns the [128, SC] psum broadcast of 1/rms."""
                    if c > 0:
                        x_tiles[c] = axp.tile([128, ND, SC], F16, tag="xchunk",
                                              name=f"xc{c}")
                        for dd in range(0, ND, 4):
                            nc.sync.dma_start(x_tiles[c][:, dd:dd + 4, :],
                                              xT_in[c][:, dd:dd + 4, :])
                    x_sb = x_tiles[c]
                    ps_ss = apsum.tile([1, SC], F32, tag="ss", bufs=2)
                    for dd in range(0, ND, 4):
                        xsq = awk.tile([128, 4, SC], F16, tag="xsq")
                        nc.scalar.square(xsq[:], x_sb[:, dd:dd + 4, :])
                        for d in range(4):
                            nc.tensor.matmul(
                                ps_ss[:], ones_sb[:, 0:1], xsq[:, d, :],
                                start=(dd + d == 0), stop=(dd + d == ND - 1),
                            )
                    rcp = awk.tile([1, SC], F16, tag="rcp")
                    nc.scalar.activation(
                        rcp[:], ps_ss[:], AF.Sqrt, bias=eps_sb[:], scale=1.0 / D
                    )
                    with nc.allow_low_precision(reason="1/rms fp16 is plenty"):
                        nc.vector.reciprocal(rcp[:], rcp[:])
                    ps_rb = apsum.tile([128, SC], F32, tag="rb", bufs=1)
                    nc.tensor.matmul(
                        ps_rb[:], ones_sb[0:1, :], rcp[:], start=True, stop=True
                    )
                    if scale_x:
                        for d in range(ND):
                            nc.vector.tensor_mul(
                                x_sb[:, d, :], x_sb[:, d, :], ps_rb[:]
                            )
                    return ps_rb

                # chunk 0: QK runs on RAW x with 1/rms folded into the rope
                # tables, so the first matmuls start right after wqk[0] lands
                ps_rb0 = stats(0, scale_x=False)
                ctr0 = awk.tile([64, SC], F32, tag="ctr0")
                srt0 = awk.tile([64, SC], F32, tag="srt0")
                nc.vector.tensor_mul(ctr0[:], ctab_sb[:, 0:SC], ps_rb0[0:64, :])
                nc.vector.tensor_mul(srt0[:], stab_sb[:, 0:SC], ps_rb0[0:64, :])

                wv_sb = awp.tile([128, ND, NV], F16)
                for c in range(4):
                    csl = slice(c * SC, (c + 1) * SC)
                    x_sb = x_tiles[c]
                    ct = ctr0 if c == 0 else ctab_sb[:, csl]
                    st = srt0 if c == 0 else stab_sb[:, csl]

                    for m in range(NQK):
                        if c == 0 and m == 0:
                            wm = wqk0
                        else:
                            wm = astr.tile([128, ND, 128], F16, tag="wqk")
                            nc.sync.dma_start(wm[:], wqk_in[m])
                        ps_qk = apsum.tile([128, SC], F32, tag="qk", bufs=3)
                        for d in range(ND):
                            nc.tensor.matmul(
                                ps_qk[:], wm[:, d, :], x_sb[:, d, :],
                                start=(d == 0), stop=(d == ND - 1),
                            )
                        if m < HC:
                            ro = q_sb[:, m, csl]
                        else:
                            ro = krT[:, m - HC, csl]
                        t1 = awk.tile([64, SC], F32, tag="t1")
                        t2 = awk.tile([64, SC], F32, tag="t2")
                        nc.vector.tensor_mul(t1[:], ps_qk[0:64, :], ct[:])
                        nc.vector.tensor_mul(t2[:], ps_qk[64:128, :], st[:])
                        nc.vector.tensor_sub(ro[0:64, :], t1[:], t2[:])
                        nc.vector.tensor_mul(t1[:], ps_qk[64:128, :], ct[:])
                        nc.vector.tensor_mul(t2[:], ps_qk[0:64, :], st[:])
                        nc.vector.tensor_add(ro[64:128, :], t1[:], t2[:])

                    if c == 0:
                        # V needs scaled x: do the deferred in-place scale now
                        for d in range(ND):
                            nc.vector.tensor_mul(
                                x_sb[:, d, :], x_sb[:, d, :], ps_rb0[:]
                            )
                        nc.sync.dma_start(wv_sb[:], wv_in[:])
                    elif c == 1:
                        nc.sync.dma_start(masks[:], mask_in[:])
                    elif c == 2:
                        nc.sync.dma_start(wout_sb[:], wout_in[:])
                        nc.sync.dma_start(xr0_sb[:], xrT_in[:, :, 0:SH])
                    if c < 3:
                        stats(c + 1)

                    for sb in range(NKBC):
                        ps_v = apsum.tile([128, NV], F32, tag="v", bufs=2)
                        tsl = slice(sb * 128, (sb + 1) * 128)
                        for d in range(ND):
                            nc.tensor.matmul(
                                ps_v[:], x_sb[:, d, tsl], wv_sb[:, d, :],
                                start=(d == 0), stop=(d == ND - 1),
                            )
                        nc.vector.tensor_copy(vtok[:, c * NKBC + sb, :], ps_v[:])

            # ========== phase B: attention + fused out_proj partials ==========
            # two q-halves; each half's out_proj partials feed their own RS
            last_b = {}  # instruction handles for cross-phase ordering edges
            cc_h = [None, None]
            with (
                tc.tile_pool(name="b_work", bufs=2) as bwk,
                tc.tile_pool(name="b_pt", bufs=3) as bpt,
                tc.tile_pool(name="b_av", bufs=2) as bav,
                tc.tile_pool(name="b_ost", bufs=2) as bost,
                tc.tile_pool(name="b_ps", bufs=2, space="PSUM") as bps,
                tc.tile_pool(name="b_ps_acc", bufs=2, space="PSUM") as bpsa,
                tc.tile_pool(name="b_ps_den", bufs=2, space="PSUM") as bpsd,
                tc.tile_pool(name="b_ps_op", bufs=2, space="PSUM") as bpso,
            ):
                for c in range(4):
                    for s in range(2):
                        qsl = slice(c * SC + s * SH, c * SC + (s + 1) * SH)
                        nkb = 4 * c + 2 * s + 2
                        npair = nkb // 2
                        avc = bav.tile([128, HC, SH], F16, tag="avc")
                        for h in range(HC):
                            ps_av = bpsa.tile([128, SH], F32, tag="av")
                            ps_den = bpsd.tile([128, SH], F32, tag="dn")
                            prev = None

                            def flush(prev):
                                pp, pi = prev
                                for j in range(2):
                                    kb = 2 * pi + j
                                    nc.tensor.matmul(
                                        ps_av[:],
                                        vtok[:, kb, h * DH:(h + 1) * DH],
                                        pp[:, j, :], start=(kb == 0),
                                        stop=(kb == nkb - 1),
                                    )
                                    nc.tensor.matmul(
                                        ps_den[:], ones_sb[:], pp[:, j, :],
                                        start=(kb == 0), stop=(kb == nkb - 1),
                                    )

                            for pi in range(npair):
                                pts = bpt.tile([128, 2, SH], F16, tag="pt")
                                ps_sc = bps.tile([128, 2, SH], F32, tag="sc")
                                for j in range(2):
                                    kb = 2 * pi + j
                                    nc.tensor.matmul(
                                        ps_sc[:, j, :],
                                        krT[:, h, kb * 128:(kb + 1) * 128],
                                        q_sb[:, h, qsl],
                                        start=True, stop=True,
                                    )
                                if pi == npair - 1:
                                    nc.vector.tensor_add(
                                        ps_sc[:], ps_sc[:], masks[:]
                                    )
                                nc.scalar.activation(
                                    pts[:], ps_sc[:], AF.Exp,
                                    bias=nb3_sb[:], scale=ISQ,
                                )
                                if prev is not None:
                                    flush(prev)
                                prev = (pts, pi)
                            flush(prev)
                            denb = bwk.tile([128, SH], F32, tag="denb")
                            nc.vector.reciprocal(denb[:], ps_den[:])
                            nc.vector.tensor_mul(avc[:, h, :], ps_av[:], denb[:])
                        # fused out_proj partials for this q-half
                        ost = bost.tile([128, ND, SH], RSDT, tag="ost")
                        for m in range(ND):
                            ps_op = bpso.tile([128, SH], F32, tag="op")
                            for fb in range(HC):
                                last_b["pe"] = nc.tensor.matmul(
                                    ps_op[:],
                                    wout_sb[:, fb, m * 128:(m + 1) * 128],
                                    avc[:, fb, :],
                                    start=(fb == 0), stop=(fb == HC - 1),
                                )
                            last_b["dve"] = nc.vector.tensor_copy(
                                ost[:, m, :], ps_op[:]
                            )
                        # single batched store on the ACT queue (SP queue must
                        # stay free of anything phase B produces/consumes).
                        # chunk-pair RS split: RS0 carries chunks 0-1 (pieces
                        # 2c+s) and fires ~28% into phase B; RS1 carries
                        # chunks 2-3.
                        grp = c // 2
                        last_b["actq"] = nc.scalar.dma_start(
                            rs_in_s[grp][2 * (c % 2) + s], ost[:]
                        )
                        if c == 3 and s == 0:
                            # relaxed B->D fence: phase D's ACT/DVE lead-in may
                            # interleave with the final (c3,s1) tile; RS0 is
                            # done by then (ends ~78% into B, RS0 ~72%)
                            last_b["dve_c2"] = last_b["dve"]
                            last_b["actq_c2"] = last_b["actq"]
                        if c % 2 == 1 and s == 1:
                            cc_h[grp] = nc.gpsimd.collective_compute(
                                "ReduceScatter",
                                mybir.AluOpType.add,
                                replica_groups=[[0, 1, 2, 3], [4, 5, 6, 7]],
                                ins=[rs_in_s[grp][:].opt()],
                                outs=[rs_out_s[grp][:].opt()],
                            )

            kvq_ctx.__exit__(None, None, None)

            # ============ phase D: residual + RMSNorm2 + MLP ==============
            # split into sequence-halves: half 0's FC+proj hides RS1.
            # pool open order + pad place wfc/wproj streams in phase-A's dead
            # SBUF zone so their prefetch fills every slot during phase B.
            with (
                tc.tile_pool(name="d_res", bufs=1) as dres,
                tc.tile_pool(name="d_pad", bufs=1) as dpad,
                tc.tile_pool(name="d_wfc", bufs=10) as dwfc,
                tc.tile_pool(name="d_wpr", bufs=4) as dwpr,
                tc.tile_pool(name="d_o2", bufs=8) as do2,
                tc.tile_pool(name="d_ps", bufs=2, space="PSUM") as dps,
                tc.tile_pool(name="d_ps1", bufs=2, space="PSUM") as dps1,
            ):
                pad = dpad.tile([128, 5120], F16)  # keep streams off B's zone
                gT_tiles = [
                    dres.tile([128, NFF, SH], F16, name=f"gT{s}", tag=f"gT{s}")
                    for s in range(2)
                ]
                xr1_sb = dres.tile([128, ND, SH], F16)
                nc.sync.dma_start(xr1_sb[:], xrT_in[:, :, SH:SC])
                first_d = {}
                prev_s = {}
                NPRE = 10
                for s in range(2):
                    ssl = slice(s * SH, (s + 1) * SH)
                    if s == 0:
                        xr, rsb, out1, h2T = xr0_sb, rsb0_sb, out10_sb, h2T0_sb
                    else:
                        xr = xr1_sb
                        rsb = dres.tile([128, ND, SH], RSDT, tag="rsb1")
                        out1 = dres.tile([128, ND, SH], F16, tag="out11")
                        h2T = dres.tile([128, ND, SH], F16, tag="h2T1")
                    # wfc prefetch: fill all ring slots before FC needs them
                    wms = [None] * NFF

                    def load_wfc(m, s=s):
                        wms[m] = dwfc.tile([128, ND, 128], F16, tag="wfc",
                                           name=f"wfc{s}_{m}")
                        nc.sync.dma_start(wms[m][:], wfc_in[m])

                    for m in range(NPRE):
                        load_wfc(m)

                    # RS-gated load on the GpSimd queue: nothing else needs it
                    ld = nc.gpsimd.dma_start(rsb[:], rs_out_s[s][:])
                    if s == 0:
                        # keep RS1's trigger behind the rsb0 load on the
                        # gpsimd queue (it can't fire earlier anyway)
                        add_dep_helper(cc_h[1].ins, ld.ins,
                                       reason="rs1 trigger after rsb0 load")
                    h = nc.vector.tensor_add(out1[:], rsb[:], xr[:])
                    if s == 0:
                        first_d["dve"] = h
                    else:
                        add_dep_helper(h.ins, prev_s["dve"].ins,
                                       reason="D half order dve")
                    ps_ss2 = dps1.tile([1, SH], F32, tag="ss2")
                    for dd in range(0, ND, 4):
                        xsq = dwke.tile([128, 4, SH], F16, tag="xsq2")
                        h = nc.scalar.square(xsq[:], out1[:, dd:dd + 4, :])
                        if s == 0 and dd == 0:
                            first_d["act"] = h
                        elif s == 1 and dd == 0:
                            add_dep_helper(h.ins, prev_s["act"].ins,
                                           reason="D half order act")
                        for d in range(4):
                            h = nc.tensor.matmul(
                                ps_ss2[:], ones_sb[:, 0:1], xsq[:, d, :],
                                start=(dd + d == 0), stop=(dd + d == ND - 1),
                            )
                            if s == 0 and dd + d == 0:
                                first_d["pe"] = h
                            elif s == 1 and dd + d == 0:
                                add_dep_helper(h.ins, prev_s["pe"].ins,
                                               reason="D half order pe")
                    rcp2 = dwke.tile([1, SH], F16, tag="rcp2")
                    nc.scalar.activation(
                        rcp2[:], ps_ss2[:], AF.Sqrt, bias=eps_sb[:], scale=1.0 / D
                    )
                    with nc.allow_low_precision(reason="1/rms fp16 is plenty"):
                        nc.vector.reciprocal(rcp2[:], rcp2[:])
                    ps_rb2 = dps1.tile([128, SH], F32, tag="rb2")
                    nc.tensor.matmul(
                        ps_rb2[:], ones_sb[0:1, :], rcp2[:], start=True, stop=True
                    )
                    for d in range(ND):
                        nc.vector.tensor_mul(h2T[:, d, :], out1[:, d, :], ps_rb2[:])

                    gT = gT_tiles[s]
                    for m in range(NFF):
                        ps_fc = dps.tile([128, SH], F32, tag="fc")
                        for d in range(ND):
                            nc.tensor.matmul(
                                ps_fc[:], wms[m][:, d, :], h2T[:, d, :],
                                start=(d == 0), stop=(d == ND - 1),
                            )
                        if m + NPRE < NFF:
                            load_wfc(m + NPRE)
                        gl = nc.scalar.activation(gT[:, m, :], ps_fc[:], gelu)

                    wps = [None] * ND

                    def load_wpr(m):
                        wps[m] = dwpr.tile([128, NFF, 128], F16, tag="wproj",
                                           name=f"wpr{s}_{m}")
                        nc.sync.dma_start(wps[m][:], wproj_in[m])

                    for m in range(4):
                        load_wpr(m)
                    for m in range(ND):
                        ps_pr = dps.tile([128, SH], F32, tag="pr")
                        for k in range(NFF):
                            pe = nc.tensor.matmul(
                                ps_pr[:], wps[m][:, k, :], gT[:, k, :],
                                start=(k == 0), stop=(k == NFF - 1),
                            )
                        if m + 4 < ND:
                            load_wpr(m + 4)
                        o2 = do2.tile([128, SH], F16, tag="o2")
                        dve = nc.vector.tensor_add(o2[:], ps_pr[:], out1[:, m, :])
                        nc.sync.dma_start(out_ext[m][:, ssl], o2[:])
                        if s == 0 and m == 10:
                            # half-1 lead-in may overlap the proj(0) tail:
                            # RS1 is long done by proj(0) m=10
                            mid_dve = dve
                    prev_s = {"pe": pe, "dve": mid_dve if s == 0 else dve,
                              "act": gl}

                # B->D ordering edges: nothing phase-D (all RS-gated) may be
                # emitted ahead of phase-B work in any engine queue
                add_dep_helper(first_d["pe"].ins, last_b["pe"].ins,
                               reason="D after B: tensor queue")
                add_dep_helper(first_d["dve"].ins, last_b["dve_c2"].ins,
                               reason="D after B: vector queue")
                add_dep_helper(first_d["act"].ins, last_b["actq_c2"].ins,
                               reason="D after B: scalar queue")

    nc.compile()
    return nc


def _deinterleave(w):
    """Reorder head-dim columns: evens then odds (per 128-wide head)."""
    Din, Dout = w.shape
    nh = Dout // DH
    w4 = w.reshape(Din, nh, DH // 2, 2)
    return np.concatenate([w4[..., 0], w4[..., 1]], axis=2).reshape(Din, Dout)


def prep_inputs(x, w_qkv, w_out, w_fc, w_proj, g_in, g_ff, S, D, H, FF):
    HC = H // G
    SC = S // 4
    SH = SC // 2
    ND = D // 128
    NQK = 2 * HC
    NV = HC * DH
    NFF = FF // 128

    x = np.asarray(x, np.float32)
    w_qkv = np.asarray(w_qkv, np.float32)
    w_out = np.asarray(w_out, np.float32)
    w_fc = np.asarray(w_fc, np.float32)
    w_proj = np.asarray(w_proj, np.float32)
    g_in = np.asarray(g_in, np.float32)
    g_ff = np.asarray(g_ff, np.float32)

    wq = w_qkv * g_in[:, None]

    half = DH // 2
    invf = 1.0 / (ROPE_BASE ** (2.0 * np.arange(half, dtype=np.float64) / DH))
    ang = np.arange(S, dtype=np.float64)[:, None] * invf[None, :]
    ctab = np.ascontiguousarray(np.cos(ang).T.astype(np.float32))
    stab = np.ascontiguousarray(np.sin(ang).T.astype(np.float32))

    # two per-half mask patterns: key-block at offset 0 / 128 below the
    # q-half base (q-half rows t=0..SH-1 attend keys <= base+t)
    pp = np.arange(128)[:, None]
    tt = np.arange(SH)[None, :]
    masks = np.stack([
        np.where(pp <= tt, 0.0, -1000.0),
        np.where(pp <= tt - 128, 0.0, -1000.0),
    ], axis=0).astype(np.float32)
    masks = np.ascontiguousarray(masks.transpose(1, 0, 2))

    wfc = w_fc * g_ff[:, None]
    wfc_r = np.ascontiguousarray(
        wfc.reshape(ND, 128, NFF, 128).transpose(2, 1, 0, 3)
    ).astype(np.float16)
    wproj_r = np.ascontiguousarray(
        w_proj.reshape(NFF, 128, ND, 128).transpose(2, 1, 0, 3)
    ).astype(np.float16)
    ones16 = np.ones((128, 128), np.float16)

    in_maps = []
    for core in range(NCORES):
        b, t = core // G, core % G
        xb = x[b]
        xT = np.ascontiguousarray(xb.T)
        xT_r = np.ascontiguousarray(
            xT.reshape(ND, 128, 4, SC).transpose(2, 1, 0, 3)
        ).astype(np.float16)

        # chunk-pair RS mapping: this rank's D-half h covers global tokens
        # (2h + t//2)*SC + (t%2)*SH .. +SH
        def seg(lo):
            return xb[lo:lo + SH, :].T.reshape(ND, 128, SH).transpose(1, 0, 2)

        g0 = (t // 2) * SC + (t % 2) * SH
        g1 = (2 + t // 2) * SC + (t % 2) * SH
        xrT = np.ascontiguousarray(
            np.concatenate([seg(g0), seg(g1)], axis=2)
        ).astype(np.float16)
        qcols = _deinterleave(wq[:, t * NV:(t + 1) * NV])
        kcols = _deinterleave(wq[:, D + t * NV:D + (t + 1) * NV])
        vcols = wq[:, 2 * D + t * NV:2 * D + (t + 1) * NV]
        wqk_core = np.ascontiguousarray(
            np.concatenate([qcols, kcols], axis=1)
            .reshape(ND, 128, NQK, 128).transpose(2, 1, 0, 3)
        ).astype(np.float16)
        wv_core = np.ascontiguousarray(
            vcols.reshape(ND, 128, NV).transpose(1, 0, 2)
        ).astype(np.float16)
        wout_core = np.ascontiguousarray(
            w_out[t * NV:(t + 1) * NV, :].reshape(HC, 128, D).transpose(1, 0, 2)
        ).astype(np.float16)
        in_maps.append({
            "xT": xT_r, "xrT": xrT, "wqk": wqk_core, "wv": wv_core,
            "wout": wout_core, "wfc": wfc_r, "wproj": wproj_r,
            "ctab": ctab, "stab": stab, "masks": masks, "ones": ones16,
        })
    return in_maps


def assemble(results, S, D):
    SC = S // 4
    SH = SC // 2
    y = np.zeros((2, S, D), np.float32)
    for core in range(NCORES):
        b, t = core // G, core % G
        o = results[core]["out"].reshape(D, SC)
        g0 = (t // 2) * SC + (t % 2) * SH
        g1 = (2 + t // 2) * SC + (t % 2) * SH
        y[b, g0:g0 + SH, :] = o[:, :SH].T.astype(np.float32)
        y[b, g1:g1 + SH, :] = o[:, SH:].T.astype(np.float32)
    return y


_CACHE = {}


def run(inputs, S, D, H, FF, trace=False, **kw):
    key = (S, D, H, FF)
    if key not in _CACHE:
        _CACHE[key] = build_nc(S, D, H, FF)
    nc = _CACHE[key]
    in_maps = prep_inputs(
        inputs["x"], inputs["w_qkv"], inputs["w_out"], inputs["w_fc"],
        inputs["w_proj"], inputs["g_in"], inputs["g_ff"], S, D, H, FF,
    )
    res = run_bass_kernel_spmd(nc, in_maps, list(range(NCORES)), trace=trace, **kw)
    return assemble(res.results, S, D), res


def kernel(**inputs):
    y, _ = run(inputs, S=2048, D=2048, H=16, FF=4096)
    return y.astype(np.float32)


# revision 40
# speedup vs baseline: 1.0415x; 1.0144x over previous
"""Trainium2 Bass kernel for one dense transformer block (RMSNorm -> causal
RoPE attention -> residual -> RMSNorm -> GELU MLP -> residual).

Sharding across 8 NeuronCores: 2 batch-groups (data parallel over B=2) x 4
ranks. Within a group: tensor-parallel over heads for QKV+attention, 4-rank
ReduceScatter reshards the out_proj partial sums to sequence-parallel, then
each rank runs the MLP on its own 512-token shard with full weights.

v5 structure (from trace-driven iteration; v2 baseline was ~956us):
- DMA queue discipline: SP carries only never-waiting weight/x streams and
  output stores; out_proj->rs_in stores go on the ACT queue; the RS-gated
  rs_out loads go on the GpSimd queue (so no in-order queue mixes an
  RS-gated op with work another phase needs -- the v2 bottleneck).
- Half-0 residual tiles live in a dedicated virgin-SBUF pool so their loads
  are not zone-WAR-blocked behind phase-B tile readers.
- wfc/wproj stream pools are placed (via open order + a pad pool) in phase
  A's dead SBUF zone: their prefetch fills every ring slot during phase B,
  riding out the DMA contention with the in-flight ReduceScatter.
- MLP split into sequence-halves; half 0's FC+proj hides RS1.
- Per-engine ordering edges at phase seams, targeted a few tiles before the
  seam so the next phase's lead-in chain overlaps the previous phase tail.
- exp batched over key-block pairs; psum->sbuf copies on DVE; rms stats via
  fp16 square + ones-matmul partition broadcast (no DRAM roundtrip).
- fp8 evaluated and rejected: any single fp8 matmul costs 1-2.8e-2 max-rel
  error vs the 2e-2 budget.
"""

import os
import sys

import numpy as np

for _p in ("/root/.axon_site/_ro/trn_rl_repo", "/opt/trn_rl_repo"):
    if os.path.isdir(_p) and _p not in sys.path:
        sys.path.append(_p)

import concourse.bass as bass  # noqa: E402
import concourse.mybir as mybir  # noqa: E402
import concourse.tile as tile  # noqa: E402
from concourse import bacc  # noqa: E402
from concourse.bass_utils import run_bass_kernel_spmd  # noqa: E402
from concourse.tile import add_dep_helper  # noqa: E402

F32 = mybir.dt.float32
F16 = mybir.dt.float16
AF = mybir.ActivationFunctionType

G = 4  # ranks per batch-group
NCORES = 8
DH = 128  # head dim (= partition width)
EPS = 1e-6
ROPE_BASE = 10000.0
EXPB = -3.0  # softmax exp bias


def build_nc(S, D, H, FF, gelu=None):
    gelu = gelu if gelu is not None else AF.Gelu_apprx_tanh
    HC = H // G  # heads per core
    SC = S // 4  # chunk length == sequence shard length
    SH = SC // 2  # q-half length (RS pipeline granularity)
    ND = D // 128
    NQK = 2 * HC
    NV = HC * DH
    NKBC = SC // 128  # 128-token k-blocks per chunk
    NFF = FF // 128
    ISQ = float(1.0 / np.sqrt(DH))
    RSDT = F16

    nc = bacc.Bacc("TRN2", target_bir_lowering=False, debug=False, num_devices=NCORES)

    xT_in = nc.dram_tensor("xT", [4, 128, ND, SC], F16, kind="ExternalInput")
    xrT_in = nc.dram_tensor("xrT", [128, ND, SC], F16, kind="ExternalInput")
    wqk_in = nc.dram_tensor("wqk", [NQK, 128, ND, 128], F16, kind="ExternalInput")
    wv_in = nc.dram_tensor("wv", [128, ND, NV], F16, kind="ExternalInput")
    wout_in = nc.dram_tensor("wout", [128, HC, D], F16, kind="ExternalInput")
    wfc_in = nc.dram_tensor("wfc", [NFF, 128, ND, 128], F16, kind="ExternalInput")
    wproj_in = nc.dram_tensor("wproj", [ND, 128, NFF, 128], F16, kind="ExternalInput")
    ctab_in = nc.dram_tensor("ctab", [64, S], F32, kind="ExternalInput")
    stab_in = nc.dram_tensor("stab", [64, S], F32, kind="ExternalInput")
    mask_in = nc.dram_tensor("masks", [128, 2, SH], F32, kind="ExternalInput")
    ones_in = nc.dram_tensor("ones", [128, 128], F16, kind="ExternalInput")
    out_ext = nc.dram_tensor("out", [ND, 128, SC], F16, kind="ExternalOutput")

    with tile.TileContext(nc) as tc:
        with (
            tc.tile_pool(name="const", bufs=1) as constp,
            tc.tile_pool(name="dram", bufs=1, space="DRAM") as dramp,
            tc.tile_pool(name="d_early", bufs=1) as dearly,
            tc.tile_pool(name="d_wk_e", bufs=2) as dwke,
        ):
            rs_in_s = [dramp.tile([G, 128, ND, SH], RSDT, name=f"rsi{s}",
                                  tag=f"rsi{s}") for s in range(2)]
            rs_out_s = [dramp.tile([128, ND, SH], RSDT, name=f"rso{s}",
                                   tag=f"rso{s}") for s in range(2)]

            ones_sb = constp.tile([128, 128], F16)
            nc.sync.dma_start(ones_sb[:], ones_in[:])
            eps_sb = constp.tile([1, 1], F32)
            nc.vector.memset(eps_sb[:], EPS)
            nb3_sb = constp.tile([128, 1], F32)
            nc.vector.memset(nb3_sb[:], EXPB)
            masks = constp.tile([128, 2, SH], F32)
            wout_sb = constp.tile([128, HC, D], F16)

            # half-0 residual tiles in virgin SBUF: no zone-reuse WAR against
            # A/B tiles, so their loads/writes can run during phases A/B
            xr0_sb = dearly.tile([128, ND, SH], F16)
            rsb0_sb = dearly.tile([128, ND, SH], RSDT)
            out10_sb = dearly.tile([128, ND, SH], F16)
            h2T0_sb = dearly.tile([128, ND, SH], F16)

            # q/k/v pool scoped to phases A+B; phase D reuses this SBUF
            kvq_ctx = tc.tile_pool(name="kvq", bufs=1)
            kvqp = kvq_ctx.__enter__()
            krT = kvqp.tile([128, HC, S], F16)
            q_sb = kvqp.tile([128, HC, S], F16)
            vtok = kvqp.tile([128, S // 128, NV], F16)

            # ================= phase A: stats + QKV + RoPE =================
            with (
                tc.tile_pool(name="a_w", bufs=1) as awp,
                tc.tile_pool(name="a_str", bufs=3) as astr,
                tc.tile_pool(name="a_x", bufs=2) as axp,
                tc.tile_pool(name="a_work", bufs=2) as awk,
                tc.tile_pool(name="a_ps", bufs=1, space="PSUM") as apsum,
            ):
                # startup-critical loads first: x(0) in d-quarters so the
                # stats chain starts after the first 512KB, then rope tables
                x_tiles = [None] * 4
                x_tiles[0] = axp.tile([128, ND, SC], F16, tag="xchunk",
                                      name="xc0")
                for dd in range(0, ND, 4):
                    nc.sync.dma_start(x_tiles[0][:, dd:dd + 4, :],
                                      xT_in[0][:, dd:dd + 4, :])
                wqk0 = astr.tile([128, ND, 128], F16, tag="wqk", name="wqk0")
                nc.sync.dma_start(wqk0[:], wqk_in[0])
                ctab_sb = awp.tile([64, S], F32)
                stab_sb = awp.tile([64, S], F32)
                nc.sync.dma_start(ctab_sb[:], ctab_in[:])
                nc.sync.dma_start(stab_sb[:], stab_in[:])

                def stats(c, scale_x=True):
                    """Compute 1/rms for chunk c; optionally scale x in place.
                    Returns the [128, SC] psum broadcast of 1/rms."""
                    if c > 0:
                        x_tiles[c] = axp.tile([128, ND, SC], F16, tag="xchunk",
                                              name=f"xc{c}")
                        for dd in range(0, ND, 4):
                            nc.sync.dma_start(x_tiles[c][:, dd:dd + 4, :],
                                              xT_in[c][:, dd:dd + 4, :])
                    x_sb = x_tiles[c]
                    ps_ss = apsum.tile([1, SC], F32, tag="ss", bufs=2)
                    for dd in range(0, ND, 4):
                        xsq = awk.tile([128, 4, SC], F16, tag="xsq")
                        nc.scalar.square(xsq[:], x_sb[:, dd:dd + 4, :])
                        for d in range(4):
                            nc.tensor.matmul(
                                ps_ss[:], ones_sb[:, 0:1], xsq[:, d, :],
                                start=(dd + d == 0), stop=(dd + d == ND - 1),
                            )
                    rcp = awk.tile([1, SC], F16, tag="rcp")
                    nc.scalar.activation(
                        rcp[:], ps_ss[:], AF.Sqrt, bias=eps_sb[:], scale=1.0 / D
                    )
                    with nc.allow_low_precision(reason="1/rms fp16 is plenty"):
                        nc.vector.reciprocal(rcp[:], rcp[:])
                    ps_rb = apsum.tile([128, SC], F32, tag="rb", bufs=1)
                    nc.tensor.matmul(
                        ps_rb[:], ones_sb[0:1, :], rcp[:], start=True, stop=True
                    )
                    if scale_x:
                        for d in range(ND):
                            nc.vector.tensor_mul(
                                x_sb[:, d, :], x_sb[:, d, :], ps_rb[:]
                            )
                    return ps_rb

                # chunk 0: QK runs on RAW x with 1/rms folded into the rope
                # tables, so the first matmuls start right after wqk[0] lands
                ps_rb0 = stats(0, scale_x=False)
                ctr0 = awk.tile([64, SC], F32, tag="ctr0")
                srt0 = awk.tile([64, SC], F32, tag="srt0")
                nc.vector.tensor_mul(ctr0[:], ctab_sb[:, 0:SC], ps_rb0[0:64, :])
                nc.vector.tensor_mul(srt0[:], stab_sb[:, 0:SC], ps_rb0[0:64, :])

                wv_sb = awp.tile([128, ND, NV], F16)
                for c in range(4):
                    csl = slice(c * SC, (c + 1) * SC)
                    x_sb = x_tiles[c]
                    ct = ctr0 if c == 0 else ctab_sb[:, csl]
                    st = srt0 if c == 0 else stab_sb[:, csl]

                    for m in range(NQK):
                        if c == 0 and m == 0:
                            wm = wqk0
                        else:
                            wm = astr.tile([128, ND, 128], F16, tag="wqk")
                            nc.sync.dma_start(wm[:], wqk_in[m])
                        ps_qk = apsum.tile([128, SC], F32, tag="qk", bufs=3)
                        for d in range(ND):
                            nc.tensor.matmul(
                                ps_qk[:], wm[:, d, :], x_sb[:, d, :],
                                start=(d == 0), stop=(d == ND - 1),
                            )
                        if m < HC:
                            ro = q_sb[:, m, csl]
                        else:
                            ro = krT[:, m - HC, csl]
                        t1 = awk.tile([64, SC], F32, tag="t1")
                        t2 = awk.tile([64, SC], F32, tag="t2")
                        nc.vector.tensor_mul(t1[:], ps_qk[0:64, :], ct[:])
                        nc.vector.tensor_mul(t2[:], ps_qk[64:128, :], st[:])
                        nc.vector.tensor_sub(ro[0:64, :], t1[:], t2[:])
                        nc.vector.tensor_mul(t1[:], ps_qk[64:128, :], ct[:])
                        nc.vector.tensor_mul(t2[:], ps_qk[0:64, :], st[:])
                        nc.vector.tensor_add(ro[64:128, :], t1[:], t2[:])
                        if m == 5 and c < 3 and c > 0:
                            # issue next chunk's stats early enough that its
                            # scale muls finish before QK(c+1) begins
                            stats(c + 1)

                    if c == 0:
                        # V needs scaled x: do the deferred in-place scale now
                        for d in range(ND):
                            nc.vector.tensor_mul(
                                x_sb[:, d, :], x_sb[:, d, :], ps_rb0[:]
                            )
                        nc.sync.dma_start(wv_sb[:], wv_in[:])
                    elif c == 1:
                        nc.sync.dma_start(masks[:], mask_in[:])
                    elif c == 2:
                        nc.sync.dma_start(wout_sb[:], wout_in[:])
                        nc.sync.dma_start(xr0_sb[:], xrT_in[:, :, 0:SH])
                    if c == 0:
                        stats(1)

                    for sb in range(NKBC):
                        ps_v = apsum.tile([128, NV], F32, tag="v", bufs=2)
                        tsl = slice(sb * 128, (sb + 1) * 128)
                        for d in range(ND):
                            nc.tensor.matmul(
                                ps_v[:], x_sb[:, d, tsl], wv_sb[:, d, :],
                                start=(d == 0), stop=(d == ND - 1),
                            )
                        nc.vector.tensor_copy(vtok[:, c * NKBC + sb, :], ps_v[:])

            # ========== phase B: attention + fused out_proj partials ==========
            # two q-halves; each half's out_proj partials feed their own RS
            last_b = {}  # instruction handles for cross-phase ordering edges
            cc_h = [None, None]
            with (
                tc.tile_pool(name="b_work", bufs=2) as bwk,
                tc.tile_pool(name="b_pt", bufs=3) as bpt,
                tc.tile_pool(name="b_av", bufs=2) as bav,
                tc.tile_pool(name="b_ost", bufs=2) as bost,
                tc.tile_pool(name="b_ps", bufs=2, space="PSUM") as bps,
                tc.tile_pool(name="b_ps_acc", bufs=2, space="PSUM") as bpsa,
                tc.tile_pool(name="b_ps_den", bufs=2, space="PSUM") as bpsd,
                tc.tile_pool(name="b_ps_op", bufs=2, space="PSUM") as bpso,
            ):
                for c in range(4):
                    for s in range(2):
                        qsl = slice(c * SC + s * SH, c * SC + (s + 1) * SH)
                        nkb = 4 * c + 2 * s + 2
                        npair = nkb // 2
                        avc = bav.tile([128, HC, SH], F16, tag="avc")
                        for h in range(HC):
                            ps_av = bpsa.tile([128, SH], F32, tag="av")
                            ptsum = bwk.tile([128, SH], F16, tag="ptsum")
                            prev = None

                            def flush(prev):
                                pp, pi = prev
                                for j in range(2):
                                    kb = 2 * pi + j
                                    nc.tensor.matmul(
                                        ps_av[:],
                                        vtok[:, kb, h * DH:(h + 1) * DH],
                                        pp[:, j, :], start=(kb == 0),
                                        stop=(kb == nkb - 1),
                                    )

                            for pi in range(npair):
                                pts = bpt.tile([128, 2, SH], F16, tag="pt")
                                ps_sc = bps.tile([128, 2, SH], F32, tag="sc")
                                for j in range(2):
                                    kb = 2 * pi + j
                                    nc.tensor.matmul(
                                        ps_sc[:, j, :],
                                        krT[:, h, kb * 128:(kb + 1) * 128],
                                        q_sb[:, h, qsl],
                                        start=True, stop=True,
                                    )
                                if pi == npair - 1:
                                    nc.vector.tensor_add(
                                        ps_sc[:], ps_sc[:], masks[:]
                                    )
                                nc.scalar.activation(
                                    pts[:], ps_sc[:], AF.Exp,
                                    bias=nb3_sb[:], scale=ISQ,
                                )
                                # denominator: accumulate exp-sums on DVE (the
                                # per-pair ones-matmuls cost 33us of PE)
                                with nc.allow_low_precision(reason="den f16"):
                                    if pi == 0:
                                        nc.vector.tensor_add(
                                            ptsum[:], pts[:, 0, :], pts[:, 1, :]
                                        )
                                    else:
                                        nc.vector.tensor_add(
                                            ptsum[:], ptsum[:], pts[:, 0, :]
                                        )
                                        nc.vector.tensor_add(
                                            ptsum[:], ptsum[:], pts[:, 1, :]
                                        )
                                if prev is not None:
                                    flush(prev)
                                prev = (pts, pi)
                            flush(prev)
                            ps_den = bpsd.tile([128, SH], F32, tag="dn")
                            nc.tensor.matmul(
                                ps_den[:], ones_sb[:], ptsum[:],
                                start=True, stop=True,
                            )
                            denb = bwk.tile([128, SH], F16, tag="denb")
                            with nc.allow_low_precision(reason="1/den f16"):
                                nc.vector.reciprocal(denb[:], ps_den[:])
                            nc.vector.tensor_mul(avc[:, h, :], ps_av[:], denb[:])
                        # fused out_proj partials for this q-half; 2 m-blocks
                        # share one psum bank so one DVE copy moves both
                        ost = bost.tile([128, ND, SH], RSDT, tag="ost")
                        for mg in range(ND // 2):
                            ps_op = bpso.tile([128, 2, SH], F32, tag="op")
                            for mi in range(2):
                                m = 2 * mg + mi
                                for fb in range(HC):
                                    last_b["pe"] = nc.tensor.matmul(
                                        ps_op[:, mi, :],
                                        wout_sb[:, fb, m * 128:(m + 1) * 128],
                                        avc[:, fb, :],
                                        start=(fb == 0), stop=(fb == HC - 1),
                                    )
                            last_b["dve"] = nc.vector.tensor_copy(
                                ost[:, 2 * mg:2 * mg + 2, :], ps_op[:]
                            )
                        # single batched store on the ACT queue (SP queue must
                        # stay free of anything phase B produces/consumes).
                        # chunk-pair RS split: RS0 carries chunks 0-1 (pieces
                        # 2c+s) and fires ~28% into phase B; RS1 carries
                        # chunks 2-3.
                        grp = c // 2
                        last_b["actq"] = nc.scalar.dma_start(
                            rs_in_s[grp][2 * (c % 2) + s], ost[:]
                        )
                        if c == 3 and s == 0:
                            # relaxed B->D fence: phase D's ACT/DVE lead-in may
                            # interleave with the final (c3,s1) tile; RS0 is
                            # done by then (ends ~78% into B, RS0 ~72%)
                            last_b["dve_c2"] = last_b["dve"]
                            last_b["actq_c2"] = last_b["actq"]
                        if c % 2 == 1 and s == 1:
                            cc_h[grp] = nc.gpsimd.collective_compute(
                                "ReduceScatter",
                                mybir.AluOpType.add,
                                replica_groups=[[0, 1, 2, 3], [4, 5, 6, 7]],
                                ins=[rs_in_s[grp][:].opt()],
                                outs=[rs_out_s[grp][:].opt()],
                            )

            kvq_ctx.__exit__(None, None, None)

            # ============ phase D: residual + RMSNorm2 + MLP ==============
            # split into sequence-halves: half 0's FC+proj hides RS1.
            # pool open order + pad place wfc/wproj streams in phase-A's dead
            # SBUF zone so their prefetch fills every slot during phase B.
            with (
                tc.tile_pool(name="d_res", bufs=1) as dres,
                tc.tile_pool(name="d_pad", bufs=1) as dpad,
                tc.tile_pool(name="d_wfc", bufs=10) as dwfc,
                tc.tile_pool(name="d_wpr", bufs=4) as dwpr,
                tc.tile_pool(name="d_o2", bufs=8) as do2,
                tc.tile_pool(name="d_ps", bufs=2, space="PSUM") as dps,
                tc.tile_pool(name="d_ps1", bufs=2, space="PSUM") as dps1,
            ):
                pad = dpad.tile([128, 5120], F16)  # keep streams off B's zone
                gT_tiles = [
                    dres.tile([128, NFF, SH], F16, name=f"gT{s}", tag=f"gT{s}")
                    for s in range(2)
                ]
                xr1_sb = dres.tile([128, ND, SH], F16)
                nc.sync.dma_start(xr1_sb[:], xrT_in[:, :, SH:SC])
                first_d = {}
                prev_s = {}
                NPRE = 10
                for s in range(2):
                    ssl = slice(s * SH, (s + 1) * SH)
                    if s == 0:
                        xr, rsb, out1, h2T = xr0_sb, rsb0_sb, out10_sb, h2T0_sb
                    else:
                        xr = xr1_sb
                        rsb = dres.tile([128, ND, SH], RSDT, tag="rsb1")
                        out1 = dres.tile([128, ND, SH], F16, tag="out11")
                        h2T = dres.tile([128, ND, SH], F16, tag="h2T1")
                    # wfc prefetch: fill all ring slots before FC needs them
                    wms = [None] * NFF

                    def load_wfc(m, s=s):
                        wms[m] = dwfc.tile([128, ND, 128], F16, tag="wfc",
                                           name=f"wfc{s}_{m}")
                        nc.sync.dma_start(wms[m][:], wfc_in[m])

                    for m in range(NPRE):
                        load_wfc(m)

                    # RS-gated load on the GpSimd queue: nothing else needs it
                    ld = nc.gpsimd.dma_start(rsb[:], rs_out_s[s][:])
                    if s == 0:
                        # keep RS1's trigger behind the rsb0 load on the
                        # gpsimd queue (it can't fire earlier anyway)
                        add_dep_helper(cc_h[1].ins, ld.ins,
                                       reason="rs1 trigger after rsb0 load")
                    h = nc.vector.tensor_add(out1[:], rsb[:], xr[:])
                    if s == 0:
                        first_d["dve"] = h
                    else:
                        add_dep_helper(h.ins, prev_s["dve"].ins,
                                       reason="D half order dve")
                    ps_ss2 = dps1.tile([1, SH], F32, tag="ss2")
                    for dd in range(0, ND, 4):
                        xsq = dwke.tile([128, 4, SH], F16, tag="xsq2")
                        h = nc.scalar.square(xsq[:], out1[:, dd:dd + 4, :])
                        if s == 0 and dd == 0:
                            first_d["act"] = h
                        elif s == 1 and dd == 0:
                            add_dep_helper(h.ins, prev_s["act"].ins,
                                           reason="D half order act")
                        for d in range(4):
                            h = nc.tensor.matmul(
                                ps_ss2[:], ones_sb[:, 0:1], xsq[:, d, :],
                                start=(dd + d == 0), stop=(dd + d == ND - 1),
                            )
                            if s == 0 and dd + d == 0:
                                first_d["pe"] = h
                            elif s == 1 and dd + d == 0:
                                add_dep_helper(h.ins, prev_s["pe"].ins,
                                               reason="D half order pe")
                    rcp2 = dwke.tile([1, SH], F16, tag="rcp2")
                    nc.scalar.activation(
                        rcp2[:], ps_ss2[:], AF.Sqrt, bias=eps_sb[:], scale=1.0 / D
                    )
                    with nc.allow_low_precision(reason="1/rms fp16 is plenty"):
                        nc.vector.reciprocal(rcp2[:], rcp2[:])
                    ps_rb2 = dps1.tile([128, SH], F32, tag="rb2")
                    nc.tensor.matmul(
                        ps_rb2[:], ones_sb[0:1, :], rcp2[:], start=True, stop=True
                    )
                    for d in range(ND):
                        nc.vector.tensor_mul(h2T[:, d, :], out1[:, d, :], ps_rb2[:])

                    gT = gT_tiles[s]
                    for m in range(NFF):
                        ps_fc = dps.tile([128, SH], F32, tag="fc")
                        for d in range(ND):
                            nc.tensor.matmul(
                                ps_fc[:], wms[m][:, d, :], h2T[:, d, :],
                                start=(d == 0), stop=(d == ND - 1),
                            )
                        if m + NPRE < NFF:
                            load_wfc(m + NPRE)
                        gl = nc.scalar.activation(gT[:, m, :], ps_fc[:], gelu)

                    wps = [None] * ND

                    def load_wpr(m):
                        wps[m] = dwpr.tile([128, NFF, 128], F16, tag="wproj",
                                           name=f"wpr{s}_{m}")
                        nc.sync.dma_start(wps[m][:], wproj_in[m])

                    for m in range(4):
                        load_wpr(m)
                    for m in range(ND):
                        ps_pr = dps.tile([128, SH], F32, tag="pr")
                        for k in range(NFF):
                            pe = nc.tensor.matmul(
                                ps_pr[:], wps[m][:, k, :], gT[:, k, :],
                                start=(k == 0), stop=(k == NFF - 1),
                            )
                        if m + 4 < ND:
                            load_wpr(m + 4)
                        o2 = do2.tile([128, SH], F16, tag="o2")
                        dve = nc.vector.tensor_add(o2[:], ps_pr[:], out1[:, m, :])
                        nc.sync.dma_start(out_ext[m][:, ssl], o2[:])
                        if s == 0 and m == 10:
                            # half-1 lead-in may overlap the proj(0) tail:
                            # RS1 is long done by proj(0) m=10
                            mid_dve = dve
                    prev_s = {"pe": pe, "dve": mid_dve if s == 0 else dve,
                              "act": gl}

                # B->D ordering edges: nothing phase-D (all RS-gated) may be
                # emitted ahead of phase-B work in any engine queue
                add_dep_helper(first_d["pe"].ins, last_b["pe"].ins,
                               reason="D after B: tensor queue")
                add_dep_helper(first_d["dve"].ins, last_b["dve_c2"].ins,
                               reason="D after B: vector queue")
                add_dep_helper(first_d["act"].ins, last_b["actq_c2"].ins,
                               reason="D after B: scalar queue")

    nc.compile()
    return nc


def _deinterleave(w):
    """Reorder head-dim columns: evens then odds (per 128-wide head)."""
    Din, Dout = w.shape
    nh = Dout // DH
    w4 = w.reshape(Din, nh, DH // 2, 2)
    return np.concatenate([w4[..., 0], w4[..., 1]], axis=2).reshape(Din, Dout)


def prep_inputs(x, w_qkv, w_out, w_fc, w_proj, g_in, g_ff, S, D, H, FF):
    HC = H // G
    SC = S // 4
    SH = SC // 2
    ND = D // 128
    NQK = 2 * HC
    NV = HC * DH
    NFF = FF // 128

    x = np.asarray(x, np.float32)
    w_qkv = np.asarray(w_qkv, np.float32)
    w_out = np.asarray(w_out, np.float32)
    w_fc = np.asarray(w_fc, np.float32)
    w_proj = np.asarray(w_proj, np.float32)
    g_in = np.asarray(g_in, np.float32)
    g_ff = np.asarray(g_ff, np.float32)

    wq = w_qkv * g_in[:, None]

    half = DH // 2
    invf = 1.0 / (ROPE_BASE ** (2.0 * np.arange(half, dtype=np.float64) / DH))
    ang = np.arange(S, dtype=np.float64)[:, None] * invf[None, :]
    ctab = np.ascontiguousarray(np.cos(ang).T.astype(np.float32))
    stab = np.ascontiguousarray(np.sin(ang).T.astype(np.float32))

    # two per-half mask patterns: key-block at offset 0 / 128 below the
    # q-half base (q-half rows t=0..SH-1 attend keys <= base+t)
    pp = np.arange(128)[:, None]
    tt = np.arange(SH)[None, :]
    masks = np.stack([
        np.where(pp <= tt, 0.0, -1000.0),
        np.where(pp <= tt - 128, 0.0, -1000.0),
    ], axis=0).astype(np.float32)
    masks = np.ascontiguousarray(masks.transpose(1, 0, 2))

    wfc = w_fc * g_ff[:, None]
    wfc_r = np.ascontiguousarray(
        wfc.reshape(ND, 128, NFF, 128).transpose(2, 1, 0, 3)
    ).astype(np.float16)
    wproj_r = np.ascontiguousarray(
        w_proj.reshape(NFF, 128, ND, 128).transpose(2, 1, 0, 3)
    ).astype(np.float16)
    ones16 = np.ones((128, 128), np.float16)

    in_maps = []
    for core in range(NCORES):
        b, t = core // G, core % G
        xb = x[b]
        xT = np.ascontiguousarray(xb.T)
        xT_r = np.ascontiguousarray(
            xT.reshape(ND, 128, 4, SC).transpose(2, 1, 0, 3)
        ).astype(np.float16)

        # chunk-pair RS mapping: this rank's D-half h covers global tokens
        # (2h + t//2)*SC + (t%2)*SH .. +SH
        def seg(lo):
            return xb[lo:lo + SH, :].T.reshape(ND, 128, SH).transpose(1, 0, 2)

        g0 = (t // 2) * SC + (t % 2) * SH
        g1 = (2 + t // 2) * SC + (t % 2) * SH
        xrT = np.ascontiguousarray(
            np.concatenate([seg(g0), seg(g1)], axis=2)
        ).astype(np.float16)
        qcols = _deinterleave(wq[:, t * NV:(t + 1) * NV])
        kcols = _deinterleave(wq[:, D + t * NV:D + (t + 1) * NV])
        vcols = wq[:, 2 * D + t * NV:2 * D + (t + 1) * NV]
        wqk_core = np.ascontiguousarray(
            np.concatenate([qcols, kcols], axis=1)
            .reshape(ND, 128, NQK, 128).transpose(2, 1, 0, 3)
        ).astype(np.float16)
        wv_core = np.ascontiguousarray(
            vcols.reshape(ND, 128, NV).transpose(1, 0, 2)
        ).astype(np.float16)
        wout_core = np.ascontiguousarray(
            w_out[t * NV:(t + 1) * NV, :].reshape(HC, 128, D).transpose(1, 0, 2)
        ).astype(np.float16)
        in_maps.append({
            "xT": xT_r, "xrT": xrT, "wqk": wqk_core, "wv": wv_core,
            "wout": wout_core, "wfc": wfc_r, "wproj": wproj_r,
            "ctab": ctab, "stab": stab, "masks": masks, "ones": ones16,
        })
    return in_maps


def assemble(results, S, D):
    SC = S // 4
    SH = SC // 2
    y = np.zeros((2, S, D), np.float32)
    for core in range(NCORES):
        b, t = core // G, core % G
        o = results[core]["out"].reshape(D, SC)
        g0 = (t // 2) * SC + (t % 2) * SH
        g1 = (2 + t // 2) * SC + (t % 2) * SH
        y[b, g0:g0 + SH, :] = o[:, :SH].T.astype(np.float32)
        y[b, g1:g1 + SH, :] = o[:, SH:].T.astype(np.float32)
    return y


_CACHE = {}


def run(inputs, S, D, H, FF, trace=False, **kw):
    key = (S, D, H, FF)
    if key not in _CACHE:
        _CACHE[key] = build_nc(S, D, H, FF)
    nc = _CACHE[key]
    in_maps = prep_inputs(
        inputs["x"], inputs["w_qkv"], inputs["w_out"], inputs["w_fc"],
        inputs["w_proj"], inputs["g_in"], inputs["g_ff"], S, D, H, FF,
    )
    res = run_bass_kernel_spmd(nc, in_maps, list(range(NCORES)), trace=trace, **kw)
    return assemble(res.results, S, D), res


def kernel(**inputs):
    y, _ = run(inputs, S=2048, D=2048, H=16, FF=4096)
    return y.astype(np.float32)
